# revision 1
# baseline (speedup 1.0000x reference)
"""nn_AttentionConv_71339406787062 — 8-core batch-data-parallel kernel.

Strategy (per spec sharding_hint): data-parallel over batch B=8 across the 8
NeuronCores — one batch element per core. Every stage is batch-independent
except the training-mode BatchNorm statistics, which are computed with a
cross-core psum inside the SPMD program.

The reference's dense [B,G,N,N] scatter + sum(axis=2) is algebraically
replaced by a dedup-masked scatter-add into the [G,N] score histogram:
`.at[].set` keeps exactly one value per (n, j) bin — the LAST m wins (verified
against the XLA backend) — so zeroing every m that has a later duplicate m'
with the same idx in its row and then scatter-ADDING is bit-equivalent, and
never materializes the N x N matrix.
"""

import numpy as np

B, CIN, N, M = 8, 128, 1024, 16
G = 8
L, MC, NL = 96, 64, 32
BN_EPS = 1e-5

_compiled = None


def _build():
    import jax
    import jax.numpy as jnp

    HI = jax.lax.Precision.HIGHEST

    def per_core(x, abs_x, idx, Wq, Wk, Wv, Wmq, Wmk, Wmv, Wm2nl, bn_gamma, bn_beta):
        # x: (CIN, N, M) for this core's batch element; idx: (N, M) int32
        Cg = L // G
        Cm = MC // G

        xf = x.reshape(CIN, N * M)
        q = jnp.matmul(Wq, xf, precision=HI).reshape(G, Cg, N, M)
        kk = jnp.matmul(Wk, xf, precision=HI).reshape(G, Cg, N, M)
        vv = jnp.matmul(Wv, xf, precision=HI).reshape(G, Cg, N, M)

        logits = (q * kk).sum(1)                      # (G, N, M)
        att = jax.nn.softmax(logits, axis=-1)         # (G, N, M)

        # last-m-wins dedup mask, then scatter-add == reference scatter-set + sum(n)
        eq = idx[:, :, None] == idx[:, None, :]       # (N, M, M)
        later = jnp.triu(jnp.ones((M, M), bool), k=1)  # m' > m
        loser = (eq & later[None]).any(axis=2)        # (N, M) True -> a later dup exists
        w = att * (~loser)[None].astype(att.dtype)    # (G, N, M)

        flat_idx = idx.reshape(-1)                    # (N*M,)
        score = jnp.zeros((G, N), att.dtype).at[:, flat_idx].add(
            w.reshape(G, N * M))                      # (G, N)

        val_score, top_idx = jax.lax.top_k(score, M)  # (G, M)

        out_l = jnp.einsum('gnm,gcnm->gcn', att, vv,
                           precision=HI).reshape(L, N)

        # --- non-local branch ---
        a = abs_x[:, :, 0]                            # (CIN//2, N)
        qm = jnp.matmul(Wmq, a, precision=HI).reshape(G, Cm, N)
        km = jnp.matmul(Wmk, a, precision=HI).reshape(G, Cm, N)
        vm = jnp.matmul(Wmv, a, precision=HI).reshape(G, Cm, N)

        gidx = jnp.broadcast_to(top_idx[:, None, :], (G, Cm, M))
        km_sel = jnp.take_along_axis(km, gidx, axis=2)              # (G, Cm, M)
        vm_sel = jnp.take_along_axis(vm, gidx, axis=2) * jnp.tanh(
            val_score)[:, None, :]

        att2 = jax.nn.softmax(
            jnp.einsum('gcn,gcm->gnm', qm, km_sel, precision=HI), axis=-1)
        out_nl = jnp.einsum('gnm,gcm->gcn', att2, vm_sel,
                            precision=HI).reshape(MC, N)

        h = jnp.matmul(Wm2nl, out_nl, precision=HI)   # (NL, N)

        # training-mode BN over (B, N): cross-core psum of per-core sums
        s1 = h.sum(axis=1)                            # (NL,)
        s2 = (h * h).sum(axis=1)                      # (NL,)
        s1 = jax.lax.psum(s1, axis_name='b')
        s2 = jax.lax.psum(s2, axis_name='b')
        cnt = float(B * N)
        mu = s1 / cnt
        var = s2 / cnt - mu * mu
        hn = bn_gamma[:, None] * (h - mu[:, None]) * jax.lax.rsqrt(
            var[:, None] + BN_EPS) + bn_beta[:, None]

        out = jnp.concatenate([out_l, hn], axis=0)[:, :, None]  # (CIN, N, 1)
        return out, km_sel, vm_sel

    return jax.pmap(per_core, axis_name='b',
                    in_axes=(0, 0, 0) + (None,) * 9)


def _get_compiled():
    global _compiled
    if _compiled is None:
        _compiled = _build()
    return _compiled


def kernel(x, abs_x, idx, k, v, Wq, Wk, Wv, Wmq, Wmk, Wmv, Wm2nl,
           bn_gamma, bn_beta):
    f = _get_compiled()
    x = np.asarray(x, np.float32)
    abs_x = np.asarray(abs_x, np.float32)
    idx32 = np.asarray(idx, np.int32).reshape(B, N, M)
    out, km_sel, vm_sel = f(
        x, abs_x, idx32,
        np.asarray(Wq, np.float32), np.asarray(Wk, np.float32),
        np.asarray(Wv, np.float32), np.asarray(Wmq, np.float32),
        np.asarray(Wmk, np.float32), np.asarray(Wmv, np.float32),
        np.asarray(Wm2nl, np.float32), np.asarray(bn_gamma, np.float32),
        np.asarray(bn_beta, np.float32))
    out = np.asarray(out)                  # (B, CIN, N, 1)
    km_sel = np.asarray(km_sel)            # (B, G, MC//G, M)
    vm_sel = np.asarray(vm_sel)
    return out, km_sel, vm_sel


# revision 2
# speedup vs baseline: 1.0107x; 1.0107x over previous
"""nn_AttentionConv_71339406787062 — 8-core batch-data-parallel kernel.

Strategy (per spec sharding_hint): data-parallel over batch B=8 across the 8
NeuronCores — one batch element per core. Every stage is batch-independent
except the training-mode BatchNorm statistics, which are computed with a
cross-core psum inside the SPMD program.

The reference's dense [B,G,N,N] scatter + sum(axis=2) is algebraically
replaced by a dedup-masked scatter-add into the [G,N] score histogram:
`.at[].set` keeps exactly one value per (n, j) bin — the LAST m wins (verified
against the XLA backend) — so zeroing every m that has a later duplicate m'
with the same idx in its row and then scatter-ADDING is bit-equivalent, and
never materializes the N x N matrix.
"""

import numpy as np

B, CIN, N, M = 8, 128, 1024, 16
G = 8
L, MC, NL = 96, 64, 32
BN_EPS = 1e-5

_compiled = None


def _build():
    import jax
    import jax.numpy as jnp

    HI = jax.lax.Precision.HIGHEST

    def per_core(x, abs_x, idx, Wq, Wk, Wv, Wmq, Wmk, Wmv, Wm2nl, bn_gamma, bn_beta):
        # x: (CIN, N, M) for this core's batch element; idx: (N, M) int32
        Cg = L // G
        Cm = MC // G

        xf = x.reshape(CIN, N * M)
        q = jnp.matmul(Wq, xf, precision=HI).reshape(G, Cg, N, M)
        kk = jnp.matmul(Wk, xf, precision=HI).reshape(G, Cg, N, M)
        vv = jnp.matmul(Wv, xf, precision=HI).reshape(G, Cg, N, M)

        logits = (q * kk).sum(1)                      # (G, N, M)
        att = jax.nn.softmax(logits, axis=-1)         # (G, N, M)

        # last-m-wins dedup mask, then scatter-add == reference scatter-set + sum(n)
        eq = idx[:, :, None] == idx[:, None, :]       # (N, M, M)
        later = jnp.triu(jnp.ones((M, M), bool), k=1)  # m' > m
        loser = (eq & later[None]).any(axis=2)        # (N, M) True -> a later dup exists
        w = att * (~loser)[None].astype(att.dtype)    # (G, N, M)

        # scatter-add as a one-hot matmul: PE-friendly, avoids XLA scatter
        flat_idx = idx.reshape(-1)                    # (N*M,)
        onehot = (flat_idx[:, None] == jnp.arange(N)[None, :]).astype(att.dtype)
        score = jnp.matmul(w.reshape(G, N * M), onehot, precision=HI)  # (G, N)

        val_score, top_idx = jax.lax.top_k(score, M)  # (G, M)

        out_l = jnp.einsum('gnm,gcnm->gcn', att, vv,
                           precision=HI).reshape(L, N)

        # --- non-local branch ---
        a = abs_x[:, :, 0]                            # (CIN//2, N)
        qm = jnp.matmul(Wmq, a, precision=HI).reshape(G, Cm, N)
        km = jnp.matmul(Wmk, a, precision=HI).reshape(G, Cm, N)
        vm = jnp.matmul(Wmv, a, precision=HI).reshape(G, Cm, N)

        gidx = jnp.broadcast_to(top_idx[:, None, :], (G, Cm, M))
        km_sel = jnp.take_along_axis(km, gidx, axis=2)              # (G, Cm, M)
        vm_sel = jnp.take_along_axis(vm, gidx, axis=2) * jnp.tanh(
            val_score)[:, None, :]

        att2 = jax.nn.softmax(
            jnp.einsum('gcn,gcm->gnm', qm, km_sel, precision=HI), axis=-1)
        out_nl = jnp.einsum('gnm,gcm->gcn', att2, vm_sel,
                            precision=HI).reshape(MC, N)

        h = jnp.matmul(Wm2nl, out_nl, precision=HI)   # (NL, N)

        # training-mode BN over (B, N): cross-core psum of per-core sums
        s1 = h.sum(axis=1)                            # (NL,)
        s2 = (h * h).sum(axis=1)                      # (NL,)
        s1 = jax.lax.psum(s1, axis_name='b')
        s2 = jax.lax.psum(s2, axis_name='b')
        cnt = float(B * N)
        mu = s1 / cnt
        var = s2 / cnt - mu * mu
        hn = bn_gamma[:, None] * (h - mu[:, None]) * jax.lax.rsqrt(
            var[:, None] + BN_EPS) + bn_beta[:, None]

        out = jnp.concatenate([out_l, hn], axis=0)[:, :, None]  # (CIN, N, 1)
        return out, km_sel, vm_sel

    return jax.pmap(per_core, axis_name='b',
                    in_axes=(0, 0, 0) + (None,) * 9)


def _get_compiled():
    global _compiled
    if _compiled is None:
        _compiled = _build()
    return _compiled


def kernel(x, abs_x, idx, k, v, Wq, Wk, Wv, Wmq, Wmk, Wmv, Wm2nl,
           bn_gamma, bn_beta):
    f = _get_compiled()
    x = np.asarray(x, np.float32)
    abs_x = np.asarray(abs_x, np.float32)
    idx32 = np.asarray(idx, np.int32).reshape(B, N, M)
    out, km_sel, vm_sel = f(
        x, abs_x, idx32,
        np.asarray(Wq, np.float32), np.asarray(Wk, np.float32),
        np.asarray(Wv, np.float32), np.asarray(Wmq, np.float32),
        np.asarray(Wmk, np.float32), np.asarray(Wmv, np.float32),
        np.asarray(Wm2nl, np.float32), np.asarray(bn_gamma, np.float32),
        np.asarray(bn_beta, np.float32))
    out = np.asarray(out)                  # (B, CIN, N, 1)
    km_sel = np.asarray(km_sel)            # (B, G, MC//G, M)
    vm_sel = np.asarray(vm_sel)
    return out, km_sel, vm_sel


# revision 4
# speedup vs baseline: 14.4969x; 14.3428x over previous
"""nn_AttentionConv_71339406787062 — 8-core batch-data-parallel kernel.

Strategy (per spec sharding_hint): data-parallel over batch B=8 across the 8
NeuronCores — one batch element per core. Every stage is batch-independent
except the training-mode BatchNorm statistics, which are computed with a
cross-core psum inside the SPMD program.

The reference's dense [B,G,N,N] scatter + sum(axis=2) is algebraically
replaced by a dedup-masked scatter-add into the [G,N] score histogram:
`.at[].set` keeps exactly one value per (n, j) bin — the LAST m wins (verified
against the XLA backend) — so zeroing every m that has a later duplicate m'
with the same idx in its row and then scatter-ADDING is bit-equivalent, and
never materializes the N x N matrix.
"""

import numpy as np

B, CIN, N, M = 8, 128, 1024, 16
G = 8
L, MC, NL = 96, 64, 32
BN_EPS = 1e-5

_compiled = None


def _build():
    import jax
    import jax.numpy as jnp

    HI = jax.lax.Precision.HIGHEST

    def per_core(x, abs_x, idx, Wq, Wk, Wv, Wmq, Wmk, Wmv, Wm2nl, bn_gamma, bn_beta):
        # x: (CIN, N, M) for this core's batch element; idx: (N, M) int32
        Cg = L // G
        Cm = MC // G

        xf = x.reshape(CIN, N * M)
        q = jnp.matmul(Wq, xf, precision=HI)          # (L, N*M)
        kk = jnp.matmul(Wk, xf, precision=HI)
        vv = jnp.matmul(Wv, xf, precision=HI)

        # group-reduce as block-diagonal matmul: avoids DVE transposes
        SEL = jnp.kron(jnp.eye(G, dtype=x.dtype),
                       jnp.ones((1, Cg), x.dtype))    # (G, L)
        prod = q * kk                                 # (L, N*M)
        logits = jnp.matmul(SEL, prod, precision=HI).reshape(G, N, M)
        att = jax.nn.softmax(logits, axis=-1)         # (G, N, M)

        # last-m-wins dedup mask, then scatter-add == reference scatter-set + sum(n)
        eq = idx[:, :, None] == idx[:, None, :]       # (N, M, M)
        later = jnp.triu(jnp.ones((M, M), bool), k=1)  # m' > m
        loser = (eq & later[None]).any(axis=2)        # (N, M) True -> a later dup exists
        w = att * (~loser)[None].astype(att.dtype)    # (G, N, M)

        # scatter-add as a one-hot matmul: PE-friendly, avoids XLA scatter
        flat_idx = idx.reshape(-1)                    # (N*M,)
        onehot = (flat_idx[:, None] == jnp.arange(N)[None, :]).astype(att.dtype)
        score = jnp.matmul(w.reshape(G, N * M), onehot, precision=HI)  # (G, N)

        val_score, top_idx = jax.lax.top_k(score, M)  # (G, M)

        # out_l: broadcast att to all Cg channels via matmul, multiply, reduce m
        attb = jnp.matmul(SEL.T, att.reshape(G, N * M),
                          precision=HI)               # (L, N*M)
        out_l = (attb.reshape(L, N, M) * vv.reshape(L, N, M)).sum(-1)  # (L, N)

        # --- non-local branch ---
        a = abs_x[:, :, 0]                            # (CIN//2, N)
        qm = jnp.matmul(Wmq, a, precision=HI).reshape(G, Cm, N)
        km = jnp.matmul(Wmk, a, precision=HI).reshape(G, Cm, N)
        vm = jnp.matmul(Wmv, a, precision=HI).reshape(G, Cm, N)

        gidx = jnp.broadcast_to(top_idx[:, None, :], (G, Cm, M))
        km_sel = jnp.take_along_axis(km, gidx, axis=2)              # (G, Cm, M)
        vm_sel = jnp.take_along_axis(vm, gidx, axis=2) * jnp.tanh(
            val_score)[:, None, :]

        att2 = jax.nn.softmax(
            jnp.einsum('gcn,gcm->gnm', qm, km_sel, precision=HI), axis=-1)
        out_nl = jnp.einsum('gnm,gcm->gcn', att2, vm_sel,
                            precision=HI).reshape(MC, N)

        h = jnp.matmul(Wm2nl, out_nl, precision=HI)   # (NL, N)

        # training-mode BN over (B, N): cross-core psum of per-core sums
        s1 = h.sum(axis=1)                            # (NL,)
        s2 = (h * h).sum(axis=1)                      # (NL,)
        s1 = jax.lax.psum(s1, axis_name='b')
        s2 = jax.lax.psum(s2, axis_name='b')
        cnt = float(B * N)
        mu = s1 / cnt
        var = s2 / cnt - mu * mu
        hn = bn_gamma[:, None] * (h - mu[:, None]) * jax.lax.rsqrt(
            var[:, None] + BN_EPS) + bn_beta[:, None]

        out = jnp.concatenate([out_l, hn], axis=0)[:, :, None]  # (CIN, N, 1)
        return out, km_sel, vm_sel

    return jax.pmap(per_core, axis_name='b',
                    in_axes=(0, 0, 0) + (None,) * 9)


def _get_compiled():
    global _compiled
    if _compiled is None:
        _compiled = _build()
    return _compiled


def kernel(x, abs_x, idx, k, v, Wq, Wk, Wv, Wmq, Wmk, Wmv, Wm2nl,
           bn_gamma, bn_beta):
    f = _get_compiled()
    x = np.asarray(x, np.float32)
    abs_x = np.asarray(abs_x, np.float32)
    idx32 = np.asarray(idx, np.int32).reshape(B, N, M)
    out, km_sel, vm_sel = f(
        x, abs_x, idx32,
        np.asarray(Wq, np.float32), np.asarray(Wk, np.float32),
        np.asarray(Wv, np.float32), np.asarray(Wmq, np.float32),
        np.asarray(Wmk, np.float32), np.asarray(Wmv, np.float32),
        np.asarray(Wm2nl, np.float32), np.asarray(bn_gamma, np.float32),
        np.asarray(bn_beta, np.float32))
    out = np.asarray(out)                  # (B, CIN, N, 1)
    km_sel = np.asarray(km_sel)            # (B, G, MC//G, M)
    vm_sel = np.asarray(vm_sel)
    return out, km_sel, vm_sel


# revision 5
# speedup vs baseline: 122.8349x; 8.4732x over previous
"""nn_AttentionConv_71339406787062 — 8-core batch-data-parallel kernel.

Strategy (per spec sharding_hint): data-parallel over batch B=8 across the 8
NeuronCores — one batch element per core. Every stage is batch-independent
except the training-mode BatchNorm statistics, which are computed with a
cross-core psum inside the SPMD program.

The reference's dense [B,G,N,N] scatter + sum(axis=2) is algebraically
replaced by a dedup-masked scatter-add into the [G,N] score histogram:
`.at[].set` keeps exactly one value per (n, j) bin — the LAST m wins (verified
against the XLA backend) — so zeroing every m that has a later duplicate m'
with the same idx in its row and then scatter-ADDING is bit-equivalent, and
never materializes the N x N matrix.
"""

import numpy as np

B, CIN, N, M = 8, 128, 1024, 16
G = 8
L, MC, NL = 96, 64, 32
BN_EPS = 1e-5

_compiled = None


def _build():
    import jax
    import jax.numpy as jnp

    HI = jax.lax.Precision.HIGHEST

    def per_core(x, abs_x, idx, Wq, Wk, Wv, Wmq, Wmk, Wmv, Wm2nl, bn_gamma, bn_beta):
        # x: (CIN, N, M) for this core's batch element; idx: (N, M) int32
        Cg = L // G
        Cm = MC // G

        xf = x.reshape(CIN, N * M)
        q = jnp.matmul(Wq, xf, precision=HI)          # (L, N*M)
        kk = jnp.matmul(Wk, xf, precision=HI)
        vv = jnp.matmul(Wv, xf, precision=HI)

        # group-reduce as block-diagonal matmul: avoids DVE transposes
        SEL = jnp.kron(jnp.eye(G, dtype=x.dtype),
                       jnp.ones((1, Cg), x.dtype))    # (G, L)
        prod = q * kk                                 # (L, N*M)
        logits = jnp.matmul(SEL, prod, precision=HI).reshape(G, N, M)
        att = jax.nn.softmax(logits, axis=-1)         # (G, N, M)

        # last-m-wins dedup mask, then scatter-add == reference scatter-set + sum(n)
        eq = idx[:, :, None] == idx[:, None, :]       # (N, M, M)
        later = jnp.triu(jnp.ones((M, M), bool), k=1)  # m' > m
        loser = (eq & later[None]).any(axis=2)        # (N, M) True -> a later dup exists
        w = att * (~loser)[None].astype(att.dtype)    # (G, N, M)

        # scatter-add as a one-hot matmul: PE-friendly, avoids XLA scatter
        flat_idx = idx.reshape(-1)                    # (N*M,)
        onehot = (flat_idx[:, None] == jnp.arange(N)[None, :]).astype(att.dtype)
        score = jnp.matmul(w.reshape(G, N * M), onehot, precision=HI)  # (G, N)

        val_score, top_idx = jax.lax.top_k(score, M)  # (G, M)

        # out_l: broadcast att to all Cg channels via matmul, multiply, reduce m
        attb = jnp.matmul(SEL.T, att.reshape(G, N * M),
                          precision=HI)               # (L, N*M)
        out_l = (attb.reshape(L, N, M) * vv.reshape(L, N, M)).sum(-1)  # (L, N)

        # --- non-local branch ---
        a = abs_x[:, :, 0]                            # (CIN//2, N)
        qm = jnp.matmul(Wmq, a, precision=HI).reshape(G, Cm, N)
        km = jnp.matmul(Wmk, a, precision=HI).reshape(G, Cm, N)
        vm = jnp.matmul(Wmv, a, precision=HI).reshape(G, Cm, N)

        gidx = jnp.broadcast_to(top_idx[:, None, :], (G, Cm, M))
        km_sel = jnp.take_along_axis(km, gidx, axis=2)              # (G, Cm, M)
        vm_sel = jnp.take_along_axis(vm, gidx, axis=2) * jnp.tanh(
            val_score)[:, None, :]

        att2 = jax.nn.softmax(
            jnp.einsum('gcn,gcm->gnm', qm, km_sel, precision=HI), axis=-1)
        out_nl = jnp.einsum('gnm,gcm->gcn', att2, vm_sel,
                            precision=HI).reshape(MC, N)

        h = jnp.matmul(Wm2nl, out_nl, precision=HI)   # (NL, N)

        # training-mode BN over (B, N): cross-core psum of per-core sums
        s1 = h.sum(axis=1)                            # (NL,)
        s2 = (h * h).sum(axis=1)                      # (NL,)
        s1 = jax.lax.psum(s1, axis_name='b')
        s2 = jax.lax.psum(s2, axis_name='b')
        cnt = float(B * N)
        mu = s1 / cnt
        var = s2 / cnt - mu * mu
        hn = bn_gamma[:, None] * (h - mu[:, None]) * jax.lax.rsqrt(
            var[:, None] + BN_EPS) + bn_beta[:, None]

        out = jnp.concatenate([out_l, hn], axis=0)[:, :, None]  # (CIN, N, 1)
        return out, km_sel, vm_sel

    return jax.pmap(per_core, axis_name='b', in_axes=0)


def _get_compiled():
    global _compiled
    if _compiled is None:
        _compiled = _build()
    return _compiled


def kernel(x, abs_x, idx, k, v, Wq, Wk, Wv, Wmq, Wmk, Wmv, Wm2nl,
           bn_gamma, bn_beta):
    f = _get_compiled()
    x = np.asarray(x, np.float32)
    abs_x = np.asarray(abs_x, np.float32)
    idx32 = np.asarray(idx, np.int32).reshape(B, N, M)
    def rep(w):
        w = np.asarray(w, np.float32)
        return np.broadcast_to(w, (B,) + w.shape)
    out, km_sel, vm_sel = f(
        x, abs_x, idx32, rep(Wq), rep(Wk), rep(Wv), rep(Wmq), rep(Wmk),
        rep(Wmv), rep(Wm2nl), rep(bn_gamma), rep(bn_beta))
    out = np.asarray(out)                  # (B, CIN, N, 1)
    km_sel = np.asarray(km_sel)            # (B, G, MC//G, M)
    vm_sel = np.asarray(vm_sel)
    return out, km_sel, vm_sel
